# revision 20
# baseline (speedup 1.0000x reference)
"""Differentiable Gaussian rasterizer on 8 Trainium2 NeuronCores.

Reference computation (see problem spec): N=512 3D Gaussians are projected to
a 256x256 image plane, evaluated as separable 2D Gaussians, depth-sorted, and
alpha-composited front-to-back.

Strategy
--------
The Gaussian footprint is separable: gauss[n,h,w] = gu[n,w] * gv[n,h], so the
exp maps are tiny ([N,W] and [N,H]).  The compositing weight for Gaussian n at
pixel p is w_n = T_n * a_n with T_n = prod_{j<n} (1 - a_j).  In log space the
cumulative product becomes a cumulative sum, which a triangular matrix-multiply
computes on the TensorEngine:

    L[n,p]   = ln(1 - a[n,p])                (ScalarE fused activation)
    S        = TRI^T @ L                     (inclusive cumsum over n, f32r MM)
    E[n,p]   = exp(S[n,p])                   (ScalarE, = transmittance after n)
    img[c,p] = sum_n E[n,p] * dc[n,c]        (f32r MM; telescoped colors)

where dc[n] = c[n+1] - c[n] (dc[last] = -c[last]); the leading c_first * 1 term
is added on the host.  Compositing is associative, so the workload is sharded
as 4 depth chunks x 2 pixel halves = 8 cores; each core returns its partial
image and chunk transmittance, and the host merges:

    img = img_0 + T_0 * (img_1 + T_1 * (img_2 + T_2 * img_3))   per pixel half.

The depth sort (stable argsort over clipped z, matching jnp.argsort) and the
shard slicing happen on the host; all per-Gaussian math, the exp maps, and the
full [N x pixels] compositing run on the NeuronCores.
"""

import os
import sys

import numpy as np

for _p in ("/opt/trn_rl_repo",):
    if _p not in sys.path and os.path.isdir(_p):
        sys.path.insert(0, _p)

from contextlib import ExitStack

from concourse import bacc, mybir, tile
from concourse.bass_utils import run_bass_kernel_spmd

_ACT_PATCHED = False


def _patch_act_tables(module_arch):
    """Reorder act_func_sets so the combined ln+exp+square set is preferred,
    eliminating per-chunk ACT table reloads. Patches both consumers: bacc's
    insert_act_table_loads (via get_activation_tables) and walrus
    (via BASS_ACT_ROOT_JSON_PATH), keeping set indices consistent."""
    global _ACT_PATCHED
    if _ACT_PATCHED:
        return
    import concourse.bacc as bacc_mod
    import concourse.hw_specs as hw_specs

    pref = "natural_log_exp_and_others"
    mine = {AF.Ln, AF.Exp, AF.Square}
    orig = hw_specs.get_activation_tables

    def _tables(arch):
        d = orig(arch)
        assert pref in d and mine <= d[pref]
        # keep set order/IDs identical to act_info.json; just stop other
        # sets from claiming our functions so one resident set serves all
        return {k: (v if k == pref else (v - mine)) for k, v in d.items()}

    bacc_mod.get_activation_tables = _tables
    _ACT_PATCHED = True

H = 256
W = 256
FOCAL = 50.0
N = 512

NCHUNK = 4          # depth chunks
NHALF = 2           # pixel (row) halves
NL = N // NCHUNK    # gaussians per core = 128
HROWS = H // NHALF  # image rows per core = 128
PIX = HROWS * W     # pixels per core = 32768
CH = 8              # image rows per inner chunk
C = CH * W          # pixels per inner chunk = 1024
NK = PIX // C       # inner chunks = 32

AF = mybir.ActivationFunctionType
OP = mybir.AluOpType
F32 = mybir.dt.float32
F32R = mybir.dt.float32r
I32 = mybir.dt.int32
BF16 = mybir.dt.bfloat16
FP16 = mybir.dt.float16

# Filled after the first call; reused so repeated kernel() calls hit the
# jax/neuronx compile cache.
_NC = None
LAST_EXEC_TIME_NS = None
LAST_RESULTS = None


def _build_nc():
    nc = bacc.Bacc("TRN2", target_bir_lowering=False, debug=False)
    if os.environ.get("RASTER_ACT_PATCH", "1") == "1":
        _patch_act_tables(nc.m.arch)

    # params columns: mx my mz sx sy opac vbase
    params = nc.dram_tensor("params", [NL, 7], F32, kind="ExternalInput").ap()
    tri = nc.dram_tensor("tri", [NL, NL], F32R, kind="ExternalInput").ap()
    dcol = nc.dram_tensor("dcol", [NL, 4], FP16, kind="ExternalInput").ap()

    # rows 0-2: rgb partial image; row 3: chunk transmittance
    out4 = nc.dram_tensor("out4", [4, PIX], F32, kind="ExternalOutput").ap()

    with tile.TileContext(nc) as tc, ExitStack() as ctx:
        const = ctx.enter_context(tc.tile_pool(name="const", bufs=1))
        apool = ctx.enter_context(tc.tile_pool(name="apool", bufs=12))
        lpool = ctx.enter_context(tc.tile_pool(name="lpool", bufs=3))
        epool = ctx.enter_context(tc.tile_pool(name="epool", bufs=2))
        opool = ctx.enter_context(tc.tile_pool(name="opool", bufs=2))
        spsum = ctx.enter_context(tc.tile_pool(name="spsum", bufs=2, space="PSUM"))
        ipsum = ctx.enter_context(tc.tile_pool(name="ipsum", bufs=2, space="PSUM"))

        def load(name, ap_dram, shape, dtype):
            t = const.tile(shape, dtype, name=name, tag=name)
            nc.sync.dma_start(t[:], ap_dram)
            return t

        params_sb = load("params_sb", params, [NL, 7], F32)
        tri_sb = load("tri_sb", tri, [NL, NL], F32R)
        dcol_sb = load("dcol_sb", dcol, [NL, 4], FP16)
        means_sb = params_sb

        warm = ipsum.tile([NL, 512], F32, tag="i", name="warm")
        for _ in range(32):
            nc.tensor.matmul(
                warm[:, :NL], lhsT=tri_sb[:], rhs=tri_sb[:], start=True, stop=True
            )

        ones = const.tile([NL, 1], F32)
        nc.vector.memset(ones[:], 1.0)
        zc = const.tile([NL, 1], F32)
        nc.vector.memset(zc[:], 0.0)
        # dummy activation: starts the (single) ACT table load immediately
        tldw = const.tile([NL, 1], F32)
        nc.scalar.activation(tldw[:], ones[:], AF.Exp, bias=zc[:], scale=1.0)

        def col(name):
            return const.tile([NL, 1], F32, name=name, tag=name)

        # z = max(mz, 0.1); rz = 1/z
        z = col("z")
        nc.vector.tensor_scalar_max(z[:], means_sb[:, 2:3], 0.1)
        rz = col("rz")
        nc.vector.reciprocal(rz[:], z[:])

        # projected centers and clipped sigmas (as reciprocals)
        pu = col("pu")
        nc.vector.tensor_scalar(pu[:], means_sb[:, 0:1], rz[:], FOCAL, OP.mult, OP.mult)
        pv = col("pv")
        nc.vector.tensor_scalar(pv[:], means_sb[:, 1:2], rz[:], FOCAL, OP.mult, OP.mult)
        su = col("su")
        nc.vector.tensor_scalar(su[:], params_sb[:, 3:4], rz[:], FOCAL, OP.mult, OP.mult)
        nc.vector.tensor_scalar_max(su[:], su[:], 0.5)
        isu = col("isu")
        nc.vector.reciprocal(isu[:], su[:])
        sv = col("sv")
        nc.vector.tensor_scalar(sv[:], params_sb[:, 4:5], rz[:], FOCAL, OP.mult, OP.mult)
        nc.vector.tensor_scalar_max(sv[:], sv[:], 0.5)
        isv = col("isv")
        nc.vector.reciprocal(isv[:], sv[:])

        # activation biases: bu = -(pu + W/2) * isu ; bv = (vbase - pv) * isv
        bu = col("bu")
        nc.vector.tensor_scalar(bu[:], pu[:], W / 2, -1.0, OP.add, OP.mult)
        nc.vector.tensor_tensor(bu[:], bu[:], isu[:], OP.mult)
        bv = col("bv")
        nc.vector.tensor_tensor(bv[:], params_sb[:, 6:7], pv[:], OP.subtract)
        nc.vector.tensor_tensor(bv[:], bv[:], isv[:], OP.mult)

        lno = col("lno")
        nc.scalar.activation(lno[:], params_sb[:, 5:6], AF.Ln, bias=zc[:], scale=1.0)

        # exp maps: gu[n,w] = opac*exp(-((w - W/2 - pu)/su)^2/2), gv[n,h] likewise
        u_i = const.tile([NL, W], I32)
        nc.gpsimd.iota(u_i[:], pattern=[[1, W]], base=0, channel_multiplier=0)
        u_f = const.tile([NL, W], F32)
        nc.vector.tensor_copy(u_f[:], u_i[:])
        h_i = const.tile([NL, HROWS], I32)
        nc.gpsimd.iota(h_i[:], pattern=[[1, HROWS]], base=0, channel_multiplier=0)
        h_f = const.tile([NL, HROWS], F32)
        nc.vector.tensor_copy(h_f[:], h_i[:])

        qu = const.tile([NL, W], F32)
        nc.scalar.activation(qu[:], u_f[:], AF.Square, bias=bu[:], scale=isu[:])
        gu = const.tile([NL, W], F32)
        nc.scalar.activation(gu[:], qu[:], AF.Exp, bias=lno[:], scale=-0.5)
        qv = const.tile([NL, HROWS], F32)
        nc.scalar.activation(qv[:], h_f[:], AF.Square, bias=bv[:], scale=isv[:])
        gv = const.tile([NL, HROWS], F32)
        nc.scalar.activation(gv[:], qv[:], AF.Exp, bias=zc[:], scale=-0.5)

        # main pipeline over NK chunks of C pixels (CH image rows each),
        # emitted with a one-stage skew so ScalarE alternates ln(k+1)/exp(k)
        # without stalling on the matmuls.
        stages = {}
        HC = C // 2  # 1024-pixel half-chunks: S/E pipeline granularity

        def stage_front(k):
            a_t = apool.tile([NL, C], F32, tag="a")
            for i in range(CH):
                h = CH * k + i
                nc.vector.tensor_scalar(
                    a_t[:, i * W:(i + 1) * W], gu[:], gv[:, h:h + 1], None, OP.mult
                )
            l_t = lpool.tile([NL, C], F32R, tag="l")
            nc.scalar.activation(l_t[:], a_t[:], AF.Ln, bias=ones[:], scale=-1.0)
            s_ts = []
            for j in range(2):
                s_t = spsum.tile([NL, HC], F32, tag="s", name=f"s_t_{k}_{j}")
                for q in range(HC // 512):
                    o = j * HC + q * 512
                    nc.tensor.matmul(
                        s_t[:, q * 512:(q + 1) * 512],
                        lhsT=tri_sb[:],
                        rhs=l_t[:, o:o + 512],
                        start=True,
                        stop=True,
                    )
                s_ts.append(s_t)
            stages[k] = s_ts

        groups = {}

        def stage_back(k):
            s_ts = stages.pop(k)
            # img quadrant packing: group = 2 chunks x 2 half-chunks
            g, jg = divmod(k, 2)
            if jg == 0:
                groups[g] = ipsum.tile([NL, HC], F32, tag="i", name=f"i_t_{g}")
            i_t = groups[g]
            for j in range(2):
                e_t = epool.tile([NL, HC], FP16, tag="e", name=f"e_t_{k}_{j}")
                nc.scalar.activation(e_t[:], s_ts[j][:], AF.Exp, bias=zc[:], scale=1.0)
                q = 2 * jg + j
                for hh in range(HC // 512):
                    nc.tensor.matmul(
                        i_t[32 * q:32 * q + 4, hh * 512:(hh + 1) * 512],
                        lhsT=dcol_sb[:],
                        rhs=e_t[:, hh * 512:(hh + 1) * 512],
                        start=True,
                        stop=True,
                        tile_position=(0, 32 * q),
                    )
            if jg == 1:
                i_full = groups.pop(g)
                o_t = opool.tile([NL, HC], F32, tag="o")
                nc.vector.tensor_copy(o_t[:], i_full[:])
                last = g == NK // 2 - 1
                for qq in range(4):
                    base = g * 2 * C + qq * HC
                    eng = nc.scalar if last and qq % 2 else nc.sync
                    eng.dma_start(
                        out4[:, base:base + HC],
                        o_t[32 * qq:32 * qq + 4, :],
                    )

        for k in range(NK + 1):
            if k < NK:
                stage_front(k)
            if k >= 1:
                stage_back(k - 1)

    nc.compile()
    return nc


def _get_nc():
    global _NC
    if _NC is None:
        _NC = _build_nc()
    return _NC


def kernel(means3d, scales, opacities, colors):
    global LAST_EXEC_TIME_NS, LAST_RESULTS

    means3d = np.asarray(means3d, np.float32)
    scales = np.asarray(scales, np.float32)
    opacities = np.asarray(opacities, np.float32)
    colors = np.asarray(colors, np.float32)

    # depth sort on clipped z (stable, matching jnp.argsort)
    z = np.maximum(means3d[:, 2], 0.1)
    order = np.argsort(z, kind="stable")
    means_s = means3d[order]
    scales_s = scales[order]
    opac_s = opacities[order]
    colors_s = colors[order]

    tri = np.triu(np.ones((NL, NL), np.float32))  # tri[k, m] = 1 for k <= m

    in_maps = []
    for c in range(NCHUNK * NHALF):
        i, j = c // NHALF, c % NHALF
        sl = slice(i * NL, (i + 1) * NL)
        cc = colors_s[sl]
        import ml_dtypes
        dc = np.zeros((NL, 4), np.float32)
        dc[:-1, :3] = cc[1:] - cc[:-1]
        dc[-1, :3] = -cc[-1]
        dc[-1, 3] = 1.0
        dc = dc.astype(np.float16)
        pars = np.concatenate(
            [
                means_s[sl],
                scales_s[sl, :2],
                opac_s[sl],
                np.full((NL, 1), j * HROWS - H / 2, np.float32),
            ],
            axis=1,
        ).astype(np.float32)
        in_maps.append({"params": pars, "tri": tri, "dcol": dc})

    nc = _get_nc()
    trace = bool(os.environ.get("RASTER_TRACE"))
    core_ids = list(range(NCHUNK * NHALF))
    res = None
    last_err = None
    for attempt in range(3):
        try:
            res = run_bass_kernel_spmd(nc, in_maps, core_ids, trace=trace)
            break
        except ModuleNotFoundError:
            trace = False
        except Exception as e:  # transient device wedge: retry
            last_err = e
            import time as _time

            _time.sleep(2.0)
    if res is None:
        res = run_bass_kernel_spmd(nc, in_maps, core_ids, trace=False)
    LAST_EXEC_TIME_NS = res.exec_time_ns
    LAST_RESULTS = res

    # host combine: img = img_0 + T_0*(img_1 + T_1*(img_2 + T_2*img_3))
    out = np.empty((H, W, 3), np.float32)
    for j in range(NHALF):
        acc = None
        for i in reversed(range(NCHUNK)):
            r = res.results[i * NHALF + j]["out4"].astype(np.float64)
            c_first = colors_s[i * NL].astype(np.float64)
            partial = r[:3] + c_first[:, None]
            if acc is None:
                acc = partial
            else:
                acc = partial + r[3:4] * acc
        out[j * HROWS:(j + 1) * HROWS] = (
            acc.reshape(3, HROWS, W).transpose(1, 2, 0).astype(np.float32)
        )
    return out
